# revision 39
# baseline (speedup 1.0000x reference)
"""Trainium2 Bass kernel for nn_MultiHeadSelfAttention_55654186222044.

Reference math (per batch b, per "slice" h of the reshaped activations):
    xs  = x[b,:,h*64:(h+1)*64]                  (T=1024, D=64)
    q_i = xs @ Wq[i].T + bq[i]   (per param set i=0..15), same k_i, v_i
    scores_i = q_i.T @ k_i / 8   (64x64, contraction over T!)
    w_i = softmax(scores_i, axis=-1)
    o_i = v_i @ w_i.T ;  cat = concat_i o_i     (T, 1024)
    out[b,h] = cat @ Wf.T + bf                  (T, 1024)

Because attention is over the feature dim, everything collapses through a
65x65 Gram matrix G = xa.T @ xa (xa = [xs, 1]):
    P2        = G @ W~q                           (65, 1024)  per slice
    scT chunk = W~k_chunk.T @ P2_chunk            (128, 4*128) one matmul per
                chunk covers ALL 4 slices; diagonal 64x64 blocks are
                scores_i^T (softmax axis lands on the psum partition dim)
    expC      = exp(scT) written into a zeroed [128,8,4,128] tile so each
                (chunk, slice) lhsT is BLOCK-DIAGONAL -> one matmul per
                (chunk, slice) computes both heads' M~^T at once:
    M~^T      = expC.T @ [Wv_aug | bv | 1]        (128, 66), last col = denom
    M         = M~ * (1/denom) per row
    N         = M.T @ Wf.T + u64 x bf             (65, 1024)  per slice
    out[b,h]  = xa @ N
This cuts FLOPs ~10x vs the naive dataflow and keeps the tensor-engine
instruction count low (matmul streaming cycles dominate). |scores| < ~50 so
exp needs no max-subtraction (f32 psum, bf16 storage). Output is stored as
fp16 (rounding ~5e-4 of absmax, well within tolerance) to halve the HBM
write traffic; the host upcasts to f32.

Sharding: 32 independent (b, h) slices; 8 cores x 4 slices. Core c takes
b = c//4 and heads 4*(c%4)..4*(c%4)+3 so its x columns are contiguous.
Weights replicated, no collectives. Emission: dense head phase (G -> P2 ->
scores -> M for all 4 slices) overlapped with the input DMAs (packed into
few large transfers split over both HWDGE queues — many small DMAs cost
~1.5us each in issue/sem latency), then the big N / out matmuls run
back-to-back (N of slice j+1 striped between out stages so the PE never
starves and the DVFS governor keeps granting full clock), with psum->sbuf
casts alternating between the Vector and Scalar engines (GPSIMD cannot read
PSUM) and the fp16 output streaming to HBM in 16 DMAs.
"""

import numpy as np
import ml_dtypes

B, T, E, H = 2, 1024, 1024, 16
D = E // H
SCALE = float(np.sqrt(D))
NCORES = 8

_CACHE = {}


def _build_nc():
    from contextlib import ExitStack

    import concourse.bass as bass
    import concourse.mybir as mybir
    import concourse.tile as tile
    from concourse import bacc

    dt = mybir.dt
    AF = mybir.ActivationFunctionType

    nc = bacc.Bacc(None)
    # packed fp16 input: cols [0:2080]=xh, [2080:3104]=wqt, [3104:4128]=wkt,
    # row 0 cols [4128:5152]=bfh, [5152:5217]=ub
    pk_d = nc.declare_dram_parameter("pk", [128, 5280], dt.float16, False)
    xt_d = nc.declare_dram_parameter("xt", [65, 4, 1024], dt.float16, False)
    wvs_d = nc.declare_dram_parameter("wvs", [128, 8, 66], dt.bfloat16, False)
    wft_d = nc.declare_dram_parameter("wft", [128, 8, 1024], dt.float16, False)
    out_d = nc.declare_dram_parameter("out", [4, 1024, 1024], dt.float16, True)

    with ExitStack() as ctx:
        tc = ctx.enter_context(tile.TileContext(nc))
        consts = ctx.enter_context(tc.tile_pool(name="consts", bufs=1))
        outp = ctx.enter_context(tc.tile_pool(name="outp", bufs=3))

        # static sbuf tensors; gpsimd memsets have no DMA deps so they run
        # from t=0 (warm first: the PE warmup depends on it)
        warm = consts.tile([128, 512], dt.float16, name="warm")
        nc.gpsimd.memset(warm[:], 0.0)
        expC = consts.tile([128, 8, 4, 128], dt.bfloat16, name="expC")
        nc.gpsimd.memset(expC[0:64], 0.0)
        nc.gpsimd.memset(expC[64:128], 0.0)

        # input DMAs: 2 on the sync HWDGE queue + 3 on the scalar HWDGE
        # queue so both drain in parallel and everything lands by ~7us
        pk = consts.tile([128, 5280], dt.float16, name="pk")
        nc.sync.dma_start(out=pk[:, 0:2080], in_=pk_d[:, 0:2080])
        nc.sync.dma_start(out=pk[:, 2080:5280], in_=pk_d[:, 2080:5280])
        wvs = consts.tile([128, 8, 66], dt.bfloat16, name="wvs")
        nc.scalar.dma_start(out=wvs[:], in_=wvs_d[:, :, :])
        wft = consts.tile([128, 8, 1024], dt.float16, name="wft")
        xt = consts.tile([65, 4, 1024], dt.float16, name="xt")
        for q in range(2):
            nc.scalar.dma_start(
                out=wft[:, 2 * q : 2 * q + 2], in_=wft_d[:, 2 * q : 2 * q + 2, :]
            )
        nc.scalar.dma_start(out=xt[:], in_=xt_d[:, :, :])
        for q in range(2, 4):
            nc.scalar.dma_start(
                out=wft[:, 2 * q : 2 * q + 2], in_=wft_d[:, 2 * q : 2 * q + 2, :]
            )

        xh = pk[:, 0:2080].rearrange("p (c j e) -> p c j e", c=8, j=4)
        wqt = pk[0:65, 2080:3104]
        wkt = pk[0:65, 3104:4128]
        bfh = pk[0:1, 4128:5152]
        ub = pk[0:1, 5152:5217]
        psb2 = consts.tile([65, 4, 1024], dt.float16, name="psb2")
        gsb = [consts.tile([65, 65], dt.float16, name=f"gsb{j}") for j in range(4)]
        msb = [consts.tile([128, 8, 65], dt.float16, name=f"msb{j}") for j in range(4)]
        nsb = [consts.tile([65, 1024], dt.float16, name=f"nsb{j}") for j in range(4)]
        rec = consts.tile([128, 8, 4], dt.float32, name="rec")

        # PE warmup: dense dummy matmuls run while the input DMAs land, so
        # the DVFS clock gate is already at 8/8 when real work starts. Also
        # preload the Exp activation table off the critical path.
        wexp = consts.tile([1, 16], dt.float16, name="wexp")
        nc.scalar.activation(out=wexp[:], in_=warm[0:1, 0:16], func=AF.Exp)
        with tc.tile_pool(name="pwarm", bufs=1, space="PSUM") as pw:
            wps = pw.tile([128, 512], dt.float32, name="warmps", tag="pw")
            for _ in range(6):
                nc.tensor.matmul(wps[:], warm[:, 0:128], warm[:], start=True, stop=True)

        # ---------------- head phase: G, P2, scores+exp, M for all slices
        with tc.tile_pool(name="pg", bufs=2, space="PSUM") as pg, \
             tc.tile_pool(name="pp0", bufs=1, space="PSUM") as pp0, \
             tc.tile_pool(name="pp1", bufs=1, space="PSUM") as pp1, \
             tc.tile_pool(name="psc", bufs=2, space="PSUM") as psc, \
             tc.tile_pool(name="pm", bufs=2, space="PSUM") as pm:
            # G_j = xa_j.T @ xa_j  (65, 65)
            for j in range(4):
                gps = pg.tile([65, 65], dt.float32, name=f"gps{j}", tag="g")
                for c in range(8):
                    nc.tensor.matmul(
                        gps[:], xh[:, c, j, :], xh[:, c, j, :],
                        start=(c == 0), stop=(c == 7),
                    )
                if j % 2 == 0:
                    nc.vector.tensor_copy(out=gsb[j][:], in_=gps[:])
                else:
                    nc.scalar.copy(out=gsb[j][:], in_=gps[:])
            # P2_j = G_j @ W~q  (65, 1024)
            for j in range(4):
                ppsa = pp0.tile([65, 512], dt.float32, name=f"pps{j}a", tag="pa")
                ppsb = pp1.tile([65, 512], dt.float32, name=f"pps{j}b", tag="pb")
                nc.tensor.matmul(ppsa[:], gsb[j][:], wqt[:, 0:512], start=True, stop=True)
                nc.tensor.matmul(ppsb[:], gsb[j][:], wqt[:, 512:1024], start=True, stop=True)
                nc.vector.tensor_copy(out=psb2[:, j, 0:512], in_=ppsa[:])
                nc.scalar.copy(out=psb2[:, j, 512:1024], in_=ppsb[:])
            # scT chunks for all 4 slices in one matmul per chunk c:
            # scp = wkt_c.T @ [P2_0 | P2_1 | P2_2 | P2_3]_c   (128, 4*128)
            for c in range(8):
                scp = psc.tile([128, 4, 128], dt.float32, name=f"scp{c}", tag="s")
                nc.tensor.matmul(
                    scp[:],
                    wkt[:, c * 128 : (c + 1) * 128],
                    psb2[:, :, c * 128 : (c + 1) * 128],
                    start=True, stop=True,
                )
                # exp of the two diagonal 64x64 blocks per slice
                nc.scalar.activation(
                    out=expC[0:64, c, :, 0:64], in_=scp[0:64, :, 0:64], func=AF.Exp
                )
                nc.scalar.activation(
                    out=expC[64:128, c, :, 64:128], in_=scp[64:128, :, 64:128], func=AF.Exp
                )
            # M~^T per (c, j): block-diag lhsT does both heads in one matmul
            for c in range(8):
                mps = pm.tile([128, 4, 128], dt.float32, name=f"mps{c}", tag="m")
                for j in range(4):
                    nc.tensor.matmul(
                        mps[:, j, 0:66], expC[:, c, j, :], wvs[:, c, :],
                        start=True, stop=True,
                    )
                nc.vector.reciprocal(out=rec[:, c, :], in_=mps[:, :, 65])
                for j in range(4):
                    if j % 2 == 0:
                        nc.vector.tensor_scalar_mul(
                            out=msb[j][:, c, :], in0=mps[:, j, 0:65],
                            scalar1=rec[:, c, j : j + 1],
                        )
                    else:
                        nc.scalar.mul(
                            out=msb[j][:, c, :], in_=mps[:, j, 0:65],
                            mul=rec[:, c, j : j + 1],
                        )

        # ---------------- tail phase: N and out, software-striped
        with tc.tile_pool(name="pnw", bufs=1, space="PSUM") as pnw, \
             tc.tile_pool(name="pnx", bufs=1, space="PSUM") as pnx, \
             tc.tile_pool(name="po", bufs=3, space="PSUM") as po:

            def emit_N(j):
                """N_j = M_j.T @ Wf.T + u64 x bf  (65, 1024), fp16 in nsb."""
                for nh in range(2):
                    pool = pnw if nh == 0 else pnx
                    nsp = pool.tile(
                        [65, 512], dt.float32, name=f"nsp{j}_{nh}", tag=f"n{nh}"
                    )
                    for c in range(8):
                        nc.tensor.matmul(
                            nsp[:], msb[j][:, c, :],
                            wft[:, c, nh * 512 : (nh + 1) * 512],
                            start=(c == 0), stop=False,
                        )
                        if c % 2 == 1:
                            yield
                    nc.tensor.matmul(
                        nsp[:], ub[:], bfh[:, nh * 512 : (nh + 1) * 512],
                        start=False, stop=True,
                    )
                    if nh == 0:
                        nc.scalar.copy(out=nsb[j][:, 0:512], in_=nsp[:])
                    else:
                        nc.vector.tensor_copy(out=nsb[j][:, 512:1024], in_=nsp[:])
                    yield

            def emit_O(j, pad=False):
                """out_j = xa_j @ N_j  (1024, 1024) fp16, streamed to HBM.
                pad=True adds WAW dummy matmuls so the tensor engine's duty
                cycle stays above the DVFS downshift threshold even when the
                psum casts pace the loop."""
                for g in range(4):
                    osb = outp.tile([128, 2, 1024], dt.float16, name=f"osb{j}_{g}", tag="osb")
                    for u in range(2):
                        c = 2 * g + u
                        ops = po.tile([128, 1024], dt.float32, name=f"ops{j}_{c}", tag="o")
                        if pad:
                            nc.tensor.matmul(
                                ops[:, 0:512], warm[0:65, 0:128], warm[0:65, 0:512],
                                start=True, stop=True,
                            )
                        nc.tensor.matmul(
                            ops[:, 0:512], xt[:, j, c * 128 : (c + 1) * 128],
                            nsb[j][:, 0:512], start=True, stop=True,
                        )
                        nc.tensor.matmul(
                            ops[:, 512:1024], xt[:, j, c * 128 : (c + 1) * 128],
                            nsb[j][:, 512:1024], start=True, stop=True,
                        )
                        if (c + j) % 2 == 0:
                            nc.vector.tensor_copy(out=osb[:, u, :], in_=ops[:])
                        else:
                            nc.scalar.copy(out=osb[:, u, :], in_=ops[:])
                        yield
                    nc.sync.dma_start(
                        out=out_d[j, g * 256 : (g + 1) * 256, :].rearrange(
                            "(u p) f -> p u f", u=2
                        ),
                        in_=osb[:],
                    )

            def drain(gen):
                for _ in gen:
                    pass

            def stripe(a, b):
                a_live, b_live = True, True
                while a_live or b_live:
                    if a_live:
                        a_live = next(a, _SENT) is not _SENT
                    if b_live:
                        b_live = next(b, _SENT) is not _SENT

            # stripe N(j+1) into out(j) so the tensor queue always has ready
            # work while nsb copies and output DMAs drain; only out(3) runs
            # bare at the end.
            drain(emit_N(0))
            stripe(emit_O(0), emit_N(1))
            stripe(emit_O(1), emit_N(2))
            stripe(emit_O(2), emit_N(3))
            drain(emit_O(3))

    nc.finalize()
    return nc


_SENT = object()


def _prep_weights(Wq, bq, Wk, bk, Wv, bv, Wf, bf):
    wqt = np.zeros((65, 1024), np.float16)
    wqt[:64] = (np.transpose(Wq, (2, 0, 1)).reshape(64, H * D) / SCALE).astype(np.float16)
    wqt[64] = (bq.reshape(H * D) / SCALE).astype(np.float16)
    wkt = np.zeros((65, 1024), np.float16)
    wkt[:64] = np.transpose(Wk, (2, 0, 1)).reshape(64, H * D).astype(np.float16)
    wkt[64] = bk.reshape(H * D).astype(np.float16)
    wva_h = np.zeros((64, 16, 66), ml_dtypes.bfloat16)
    wva_h[:, :, :64] = np.transpose(Wv, (1, 0, 2)).astype(ml_dtypes.bfloat16)
    wva_h[:, :, 64] = bv.T.astype(ml_dtypes.bfloat16)
    wva_h[:, :, 65] = 1.0
    # [128, 8, 66]: rows 0:64 = head 2c, rows 64:128 = head 2c+1 (chunk c)
    wvs = np.empty((128, 8, 66), ml_dtypes.bfloat16)
    wvs[0:64] = wva_h[:, 0::2, :]
    wvs[64:128] = wva_h[:, 1::2, :]
    wft = np.ascontiguousarray(
        Wf.T.reshape(8, 128, 1024).transpose(1, 0, 2)
    ).astype(np.float16)
    # packed weight part of pk (cols 2080:5280): wqt | wkt | bfh+ub
    pkw = np.zeros((128, 3200), np.float16)
    pkw[0:65, 0:1024] = wqt
    pkw[0:65, 1024:2048] = wkt
    pkw[0, 2048:3072] = bf.astype(np.float16)
    pkw[0, 3072 + 64] = 1.0  # ub one-hot at col 64
    return pkw, wvs, wft


def _prep_x(xs):
    """xs (1024, 256) f32 -> xh (128, 8, 4, 65) fp16 with ones col,
    xt (65, 4, 1024) fp16 with ones row."""
    x16 = xs.astype(np.float16)
    xh = np.ones((128, 8, 4, 65), np.float16)
    xh[:, :, :, :64] = x16.reshape(8, 128, 4, 64).transpose(1, 0, 2, 3)
    xt = np.ones((65, 4, 1024), np.float16)
    xt[:64] = x16.reshape(1024, 4, 64).transpose(2, 1, 0)
    return xh, xt


def _run(inputs, trace=False, tmpdir=None):
    from concourse.bass_utils import run_bass_kernel_spmd

    if "nc" not in _CACHE:
        _CACHE["nc"] = _build_nc()
    nc = _CACHE["nc"]

    x = np.ascontiguousarray(np.asarray(inputs["x"]), dtype=np.float32)
    pkw, wvs, wft = _prep_weights(
        *(np.asarray(inputs[k], dtype=np.float32) for k in
          ("Wq", "bq", "Wk", "bk", "Wv", "bv", "Wf", "bf"))
    )
    common = dict(wvs=wvs, wft=wft)
    in_maps = []
    for c in range(NCORES):
        xs = np.ascontiguousarray(x[c // 4][:, (c % 4) * 256 : (c % 4 + 1) * 256])
        xhc, xtc = _prep_x(xs)
        pk = np.empty((128, 5280), np.float16)
        pk[:, 0:2080] = xhc.reshape(128, 2080)
        pk[:, 2080:5280] = pkw
        in_maps.append(dict(pk=pk, xt=xtc, **common))

    res = run_bass_kernel_spmd(
        nc, in_maps, list(range(NCORES)), trace=trace, tmpdir=tmpdir
    )
    out = np.empty((B, H, T, E), np.float32)
    for c in range(NCORES):
        out[c // 4, 4 * (c % 4) : 4 * (c % 4) + 4] = res.results[c]["out"].astype(
            np.float32
        )
    return out, res.exec_time_ns


def kernel(**inputs) -> np.ndarray:
    out, _ = _run(inputs, trace=False)
    return out


# revision 44
# speedup vs baseline: 1.2306x; 1.2306x over previous
"""Trainium2 Bass kernel for nn_MultiHeadSelfAttention_55654186222044.

Reference math (per batch b, per "slice" h of the reshaped activations):
    xs  = x[b,:,h*64:(h+1)*64]                  (T=1024, D=64)
    q_i = xs @ Wq[i].T + bq[i]   (per param set i=0..15), same k_i, v_i
    scores_i = q_i.T @ k_i / 8   (64x64, contraction over T!)
    w_i = softmax(scores_i, axis=-1)
    o_i = v_i @ w_i.T ;  cat = concat_i o_i     (T, 1024)
    out[b,h] = cat @ Wf.T + bf                  (T, 1024)

Because attention is over the feature dim, everything collapses through a
65x65 Gram matrix G = xa.T @ xa (xa = [xs, 1]):
    P2        = G @ W~q                           (65, 1024)  per slice
    scT chunk = W~k_chunk.T @ P2_chunk            (128, 4*128) one matmul per
                chunk covers ALL 4 slices; diagonal 64x64 blocks are
                scores_i^T (softmax axis lands on the psum partition dim)
    expC      = exp(scT) written into a zeroed [128,8,4,128] tile so each
                (chunk, slice) lhsT is BLOCK-DIAGONAL -> one matmul per
                (chunk, slice) computes both heads' M~^T at once:
    M~^T      = expC.T @ [Wv_aug | bv | 1]        (128, 66), last col = denom
    M         = M~ * (1/denom) per row
    N         = M.T @ Wf.T + u64 x bf             (65, 1024)  per slice
    out[b,h]  = xa @ N
This cuts FLOPs ~10x vs the naive dataflow and keeps the tensor-engine
instruction count low (matmul streaming cycles dominate). |scores| < ~50 so
exp needs no max-subtraction (f32 psum, bf16 storage). Output is stored as
fp16 (rounding ~5e-4 of absmax, well within tolerance) to halve the HBM
write traffic; the host upcasts to f32.

Sharding: 32 independent (b, h) slices; 8 cores x 4 slices. Core c takes
b = c//4 and heads 4*(c%4)..4*(c%4)+3 so its x columns are contiguous.
Weights replicated, no collectives. Emission: dense head phase (G -> P2 ->
scores -> M for all 4 slices) overlapped with the input DMAs (packed into
few large transfers split over both HWDGE queues — many small DMAs cost
~1.5us each in issue/sem latency), then the big N / out matmuls run
back-to-back (N of slice j+1 striped between out stages so the PE never
starves and the DVFS governor keeps granting full clock), with psum->sbuf
casts alternating between the Vector and Scalar engines (GPSIMD cannot read
PSUM) and the fp16 output streaming to HBM in 16 DMAs.
"""

import numpy as np
import ml_dtypes

B, T, E, H = 2, 1024, 1024, 16
D = E // H
SCALE = float(np.sqrt(D))
NCORES = 8

_CACHE = {}


def _build_nc():
    from contextlib import ExitStack

    import concourse.bass as bass
    import concourse.mybir as mybir
    import concourse.tile as tile
    from concourse import bacc

    dt = mybir.dt
    AF = mybir.ActivationFunctionType

    nc = bacc.Bacc(None)
    # packed fp16 input: cols [0:2080]=xh, [2080:3104]=wqt, [3104:4128]=wkt,
    # row 0 cols [4128:5152]=bfh, [5152:5217]=ub
    pk_d = nc.declare_dram_parameter("pk", [128, 5280], dt.float16, False)
    xt_d = nc.declare_dram_parameter("xt", [65, 4, 1024], dt.float16, False)
    wvs_d = nc.declare_dram_parameter("wvs", [128, 8, 66], dt.bfloat16, False)
    wft_d = nc.declare_dram_parameter("wft", [128, 8, 1024], dt.float16, False)
    out_d = nc.declare_dram_parameter("out", [4, 1024, 1024], dt.float16, True)

    with ExitStack() as ctx:
        tc = ctx.enter_context(tile.TileContext(nc))
        consts = ctx.enter_context(tc.tile_pool(name="consts", bufs=1))
        outp = ctx.enter_context(tc.tile_pool(name="outp", bufs=3))

        # static sbuf tensors; gpsimd memsets have no DMA deps so they run
        # from t=0 (warm first: the PE warmup depends on it)
        warm = consts.tile([128, 512], dt.float16, name="warm")
        nc.gpsimd.memset(warm[:], 0.0)
        expC = consts.tile([128, 8, 4, 128], dt.bfloat16, name="expC")
        nc.gpsimd.memset(expC[0:64], 0.0)
        nc.gpsimd.memset(expC[64:128], 0.0)

        # input DMAs: pk (xh+wqt+wkt first so G->P2->SC never wait) + xt on
        # the sync HWDGE queue; wvs + wft chunks stream on the scalar queue
        # so the N-stage consumes wft at the DMA's own cadence
        pk = consts.tile([128, 5280], dt.float16, name="pk")
        nc.sync.dma_start(out=pk[:, 0:4128], in_=pk_d[:, 0:4128])
        nc.sync.dma_start(out=pk[:, 4128:5280], in_=pk_d[:, 4128:5280])
        wvs = consts.tile([128, 8, 66], dt.bfloat16, name="wvs")
        nc.scalar.dma_start(out=wvs[:], in_=wvs_d[:, :, :])
        wft = consts.tile([128, 8, 1024], dt.float16, name="wft")
        xt = consts.tile([65, 4, 1024], dt.float16, name="xt")
        nc.sync.dma_start(out=xt[:], in_=xt_d[:, :, :])
        for q in range(4):
            nc.scalar.dma_start(
                out=wft[:, 2 * q : 2 * q + 2], in_=wft_d[:, 2 * q : 2 * q + 2, :]
            )

        xh = pk[:, 0:2080].rearrange("p (c j e) -> p c j e", c=8, j=4)
        wqt = pk[0:65, 2080:3104]
        wkt = pk[0:65, 3104:4128]
        bfh = pk[0:1, 4128:5152]
        ub = pk[0:1, 5152:5217]
        psb2 = consts.tile([65, 4, 1024], dt.float16, name="psb2")
        gsb = [consts.tile([65, 65], dt.float16, name=f"gsb{j}") for j in range(4)]
        msb = [consts.tile([128, 8, 65], dt.float16, name=f"msb{j}") for j in range(4)]
        nsb = [consts.tile([65, 1024], dt.float16, name=f"nsb{j}") for j in range(4)]
        rec = consts.tile([128, 8, 4], dt.float32, name="rec")

        # PE warmup: dense dummy matmuls run while the input DMAs land, so
        # the DVFS clock gate is already at 8/8 when real work starts. Also
        # preload the Exp activation table off the critical path.
        wexp = consts.tile([1, 16], dt.float16, name="wexp")
        nc.scalar.activation(out=wexp[:], in_=warm[0:1, 0:16], func=AF.Exp)
        with tc.tile_pool(name="pwarm", bufs=1, space="PSUM") as pw:
            wps = pw.tile([128, 512], dt.float32, name="warmps", tag="pw")
            for _ in range(6):
                nc.tensor.matmul(wps[:], warm[:, 0:128], warm[:], start=True, stop=True)

        # ---------------- head phase: G, P2, scores+exp, M for all slices
        # (N0's first-half accumulation is fused into the M loop: it keeps
        # the PE occupied through the window where wft is still streaming in,
        # which otherwise dips occupancy and trips the DVFS governor)
        with tc.tile_pool(name="pg", bufs=1, space="PSUM") as pg, \
             tc.tile_pool(name="pp0", bufs=1, space="PSUM") as pp0, \
             tc.tile_pool(name="pp1", bufs=1, space="PSUM") as pp1, \
             tc.tile_pool(name="psc", bufs=2, space="PSUM") as psc, \
             tc.tile_pool(name="pn0", bufs=1, space="PSUM") as pn0, \
             tc.tile_pool(name="pm", bufs=2, space="PSUM") as pm:
            # G_j = xa_j.T @ xa_j  (65, 65)
            for j in range(4):
                gps = pg.tile([65, 65], dt.float32, name=f"gps{j}", tag="g")
                for c in range(8):
                    nc.tensor.matmul(
                        gps[:], xh[:, c, j, :], xh[:, c, j, :],
                        start=(c == 0), stop=(c == 7),
                    )
                if j % 2 == 0:
                    nc.vector.tensor_copy(out=gsb[j][:], in_=gps[:])
                else:
                    nc.scalar.copy(out=gsb[j][:], in_=gps[:])
            # P2_j = G_j @ W~q  (65, 1024)
            for j in range(4):
                ppsa = pp0.tile([65, 512], dt.float32, name=f"pps{j}a", tag="pa")
                ppsb = pp1.tile([65, 512], dt.float32, name=f"pps{j}b", tag="pb")
                nc.tensor.matmul(ppsa[:], gsb[j][:], wqt[:, 0:512], start=True, stop=True)
                nc.tensor.matmul(ppsb[:], gsb[j][:], wqt[:, 512:1024], start=True, stop=True)
                nc.vector.tensor_copy(out=psb2[:, j, 0:512], in_=ppsa[:])
                nc.scalar.copy(out=psb2[:, j, 512:1024], in_=ppsb[:])
            # scT chunks for all 4 slices in one matmul per chunk c:
            # scp = wkt_c.T @ [P2_0 | P2_1 | P2_2 | P2_3]_c   (128, 4*128)
            for c in range(8):
                scp = psc.tile([128, 4, 128], dt.float32, name=f"scp{c}", tag="s")
                nc.tensor.matmul(
                    scp[:],
                    wkt[:, c * 128 : (c + 1) * 128],
                    psb2[:, :, c * 128 : (c + 1) * 128],
                    start=True, stop=True,
                )
                # exp of the two diagonal 64x64 blocks per slice
                nc.scalar.activation(
                    out=expC[0:64, c, :, 0:64], in_=scp[0:64, :, 0:64], func=AF.Exp
                )
                nc.scalar.activation(
                    out=expC[64:128, c, :, 64:128], in_=scp[64:128, :, 64:128], func=AF.Exp
                )
            # M~^T per (c, j): block-diag lhsT does both heads in one matmul;
            # N0's lo-half accumulates chunk-by-chunk right behind it
            nsp00 = pn0.tile([65, 512], dt.float32, name="nsp00", tag="pn0")
            for c in range(8):
                mps = pm.tile([128, 4, 128], dt.float32, name=f"mps{c}", tag="m")
                for j in range(4):
                    nc.tensor.matmul(
                        mps[:, j, 0:66], expC[:, c, j, :], wvs[:, c, :],
                        start=True, stop=True,
                    )
                nc.vector.reciprocal(out=rec[:, c, :], in_=mps[:, :, 65])
                for j in range(4):
                    if j % 2 == 0:
                        nc.vector.tensor_scalar_mul(
                            out=msb[j][:, c, :], in0=mps[:, j, 0:65],
                            scalar1=rec[:, c, j : j + 1],
                        )
                    else:
                        nc.scalar.mul(
                            out=msb[j][:, c, :], in_=mps[:, j, 0:65],
                            mul=rec[:, c, j : j + 1],
                        )
                nc.tensor.matmul(
                    nsp00[:], msb[0][:, c, :], wft[:, c, 0:512],
                    start=(c == 0), stop=False,
                )
            nc.tensor.matmul(nsp00[:], ub[:], bfh[:, 0:512], start=False, stop=True)
            nc.scalar.copy(out=nsb[0][:, 0:512], in_=nsp00[:])

        # ---------------- tail phase: N and out, software-striped
        with tc.tile_pool(name="pnw", bufs=1, space="PSUM") as pnw, \
             tc.tile_pool(name="pnx", bufs=1, space="PSUM") as pnx, \
             tc.tile_pool(name="po", bufs=3, space="PSUM") as po:

            def emit_N(j, nhs=(0, 1)):
                """N_j = M_j.T @ Wf.T + u64 x bf  (65, 1024), fp16 in nsb."""
                for nh in nhs:
                    pool = pnw if nh == 0 else pnx
                    nsp = pool.tile(
                        [65, 512], dt.float32, name=f"nsp{j}_{nh}", tag=f"n{nh}"
                    )
                    for c in range(8):
                        nc.tensor.matmul(
                            nsp[:], msb[j][:, c, :],
                            wft[:, c, nh * 512 : (nh + 1) * 512],
                            start=(c == 0), stop=False,
                        )
                        if c % 2 == 1:
                            yield
                    nc.tensor.matmul(
                        nsp[:], ub[:], bfh[:, nh * 512 : (nh + 1) * 512],
                        start=False, stop=True,
                    )
                    if nh == 0:
                        nc.scalar.copy(out=nsb[j][:, 0:512], in_=nsp[:])
                    else:
                        nc.vector.tensor_copy(out=nsb[j][:, 512:1024], in_=nsp[:])
                    yield

            def emit_O(j, pad=False):
                """out_j = xa_j @ N_j  (1024, 1024) fp16, streamed to HBM.
                pad=True adds WAW dummy matmuls so the tensor engine's duty
                cycle stays above the DVFS downshift threshold even when the
                psum casts pace the loop."""
                for g in range(4):
                    osb = outp.tile([128, 2, 1024], dt.float16, name=f"osb{j}_{g}", tag="osb")
                    for u in range(2):
                        c = 2 * g + u
                        ops = po.tile([128, 1024], dt.float32, name=f"ops{j}_{c}", tag="o")
                        if pad:
                            nc.tensor.matmul(
                                ops[:, 0:512], warm[0:65, 0:128], warm[0:65, 0:512],
                                start=True, stop=True,
                            )
                        nc.tensor.matmul(
                            ops[:, 0:512], xt[:, j, c * 128 : (c + 1) * 128],
                            nsb[j][:, 0:512], start=True, stop=True,
                        )
                        nc.tensor.matmul(
                            ops[:, 512:1024], xt[:, j, c * 128 : (c + 1) * 128],
                            nsb[j][:, 512:1024], start=True, stop=True,
                        )
                        if (c + j) % 2 == 0:
                            nc.vector.tensor_copy(out=osb[:, u, :], in_=ops[:])
                        else:
                            nc.scalar.copy(out=osb[:, u, :], in_=ops[:])
                        yield
                    nc.sync.dma_start(
                        out=out_d[j, g * 256 : (g + 1) * 256, :].rearrange(
                            "(u p) f -> p u f", u=2
                        ),
                        in_=osb[:],
                    )

            def drain(gen):
                for _ in gen:
                    pass

            def stripe(a, b):
                a_live, b_live = True, True
                while a_live or b_live:
                    if a_live:
                        a_live = next(a, _SENT) is not _SENT
                    if b_live:
                        b_live = next(b, _SENT) is not _SENT

            # stripe N(j+1) into out(j) so the tensor queue always has ready
            # work while nsb copies and output DMAs drain; only out(3) runs
            # bare at the end.
            drain(emit_N(0, nhs=(1,)))
            stripe(emit_O(0), emit_N(1))
            stripe(emit_O(1), emit_N(2))
            stripe(emit_O(2), emit_N(3))
            drain(emit_O(3))

    nc.finalize()
    return nc


_SENT = object()


def _prep_weights(Wq, bq, Wk, bk, Wv, bv, Wf, bf):
    wqt = np.zeros((65, 1024), np.float16)
    wqt[:64] = (np.transpose(Wq, (2, 0, 1)).reshape(64, H * D) / SCALE).astype(np.float16)
    wqt[64] = (bq.reshape(H * D) / SCALE).astype(np.float16)
    wkt = np.zeros((65, 1024), np.float16)
    wkt[:64] = np.transpose(Wk, (2, 0, 1)).reshape(64, H * D).astype(np.float16)
    wkt[64] = bk.reshape(H * D).astype(np.float16)
    wva_h = np.zeros((64, 16, 66), ml_dtypes.bfloat16)
    wva_h[:, :, :64] = np.transpose(Wv, (1, 0, 2)).astype(ml_dtypes.bfloat16)
    wva_h[:, :, 64] = bv.T.astype(ml_dtypes.bfloat16)
    wva_h[:, :, 65] = 1.0
    # [128, 8, 66]: rows 0:64 = head 2c, rows 64:128 = head 2c+1 (chunk c)
    wvs = np.empty((128, 8, 66), ml_dtypes.bfloat16)
    wvs[0:64] = wva_h[:, 0::2, :]
    wvs[64:128] = wva_h[:, 1::2, :]
    wft = np.ascontiguousarray(
        Wf.T.reshape(8, 128, 1024).transpose(1, 0, 2)
    ).astype(np.float16)
    # packed weight part of pk (cols 2080:5280): wqt | wkt | bfh+ub
    pkw = np.zeros((128, 3200), np.float16)
    pkw[0:65, 0:1024] = wqt
    pkw[0:65, 1024:2048] = wkt
    pkw[0, 2048:3072] = bf.astype(np.float16)
    pkw[0, 3072 + 64] = 1.0  # ub one-hot at col 64
    return pkw, wvs, wft


def _prep_x(xs):
    """xs (1024, 256) f32 -> xh (128, 8, 4, 65) fp16 with ones col,
    xt (65, 4, 1024) fp16 with ones row."""
    x16 = xs.astype(np.float16)
    xh = np.ones((128, 8, 4, 65), np.float16)
    xh[:, :, :, :64] = x16.reshape(8, 128, 4, 64).transpose(1, 0, 2, 3)
    xt = np.ones((65, 4, 1024), np.float16)
    xt[:64] = x16.reshape(1024, 4, 64).transpose(2, 1, 0)
    return xh, xt


def _run(inputs, trace=False, tmpdir=None):
    from concourse.bass_utils import run_bass_kernel_spmd

    if "nc" not in _CACHE:
        _CACHE["nc"] = _build_nc()
    nc = _CACHE["nc"]

    x = np.ascontiguousarray(np.asarray(inputs["x"]), dtype=np.float32)
    pkw, wvs, wft = _prep_weights(
        *(np.asarray(inputs[k], dtype=np.float32) for k in
          ("Wq", "bq", "Wk", "bk", "Wv", "bv", "Wf", "bf"))
    )
    common = dict(wvs=wvs, wft=wft)
    in_maps = []
    for c in range(NCORES):
        xs = np.ascontiguousarray(x[c // 4][:, (c % 4) * 256 : (c % 4 + 1) * 256])
        xhc, xtc = _prep_x(xs)
        pk = np.empty((128, 5280), np.float16)
        pk[:, 0:2080] = xhc.reshape(128, 2080)
        pk[:, 2080:5280] = pkw
        in_maps.append(dict(pk=pk, xt=xtc, **common))

    res = run_bass_kernel_spmd(
        nc, in_maps, list(range(NCORES)), trace=trace, tmpdir=tmpdir
    )
    out = np.empty((B, H, T, E), np.float32)
    for c in range(NCORES):
        out[c // 4, 4 * (c % 4) : 4 * (c % 4) + 4] = res.results[c]["out"].astype(
            np.float32
        )
    return out, res.exec_time_ns


def kernel(**inputs) -> np.ndarray:
    out, _ = _run(inputs, trace=False)
    return out


# revision 48
# speedup vs baseline: 1.2494x; 1.0152x over previous
"""Trainium2 Bass kernel for nn_MultiHeadSelfAttention_55654186222044.

Reference math (per batch b, per "slice" h of the reshaped activations):
    xs  = x[b,:,h*64:(h+1)*64]                  (T=1024, D=64)
    q_i = xs @ Wq[i].T + bq[i]   (per param set i=0..15), same k_i, v_i
    scores_i = q_i.T @ k_i / 8   (64x64, contraction over T!)
    w_i = softmax(scores_i, axis=-1)
    o_i = v_i @ w_i.T ;  cat = concat_i o_i     (T, 1024)
    out[b,h] = cat @ Wf.T + bf                  (T, 1024)

Because attention is over the feature dim, everything collapses through a
65x65 Gram matrix G = xa.T @ xa (xa = [xs, 1]):
    P2        = G @ W~q                           (65, 1024)  per slice
    scT chunk = W~k_chunk.T @ P2_chunk            (128, 4*128) one matmul per
                chunk covers ALL 4 slices; diagonal 64x64 blocks are
                scores_i^T (softmax axis lands on the psum partition dim)
    expC      = exp(scT) written into a zeroed [128,8,4,128] tile so each
                (chunk, slice) lhsT is BLOCK-DIAGONAL -> one matmul per
                (chunk, slice) computes both heads' M~^T at once:
    M~^T      = expC.T @ [Wv_aug | bv | 1]        (128, 66), last col = denom
    M         = M~ * (1/denom) per row
    N         = M.T @ Wf.T + u64 x bf             (65, 1024)  per slice
    out[b,h]  = xa @ N
This cuts FLOPs ~10x vs the naive dataflow and keeps the tensor-engine
instruction count low (matmul streaming cycles dominate). |scores| < ~50 so
exp needs no max-subtraction (f32 psum, bf16 storage). Output is stored as
fp16 (rounding ~5e-4 of absmax, well within tolerance) to halve the HBM
write traffic; the host upcasts to f32.

Sharding: 32 independent (b, h) slices; 8 cores x 4 slices. Core c takes
b = c//4 and heads 4*(c%4)..4*(c%4)+3 so its x columns are contiguous.
Weights replicated, no collectives. Emission: dense head phase (G -> P2 ->
scores -> M for all 4 slices) overlapped with the input DMAs (packed into
few large transfers split over both HWDGE queues — many small DMAs cost
~1.5us each in issue/sem latency), then the big N / out matmuls run
back-to-back (N of slice j+1 striped between out stages so the PE never
starves and the DVFS governor keeps granting full clock), with psum->sbuf
casts alternating between the Vector and Scalar engines (GPSIMD cannot read
PSUM) and the fp16 output streaming to HBM in 16 DMAs.
"""

import numpy as np
import ml_dtypes

B, T, E, H = 2, 1024, 1024, 16
D = E // H
SCALE = float(np.sqrt(D))
NCORES = 8

_CACHE = {}


def _build_nc():
    from contextlib import ExitStack

    import concourse.bass as bass
    import concourse.mybir as mybir
    import concourse.tile as tile
    from concourse import bacc

    dt = mybir.dt
    AF = mybir.ActivationFunctionType

    nc = bacc.Bacc(None)
    # packed fp16 input: cols [0:2080]=xh, [2080:3104]=wqt, [3104:4128]=wkt,
    # row 0 cols [4128:5152]=bfh, [5152:5217]=ub
    pk_d = nc.declare_dram_parameter("pk", [128, 5280], dt.float16, False)
    xt_d = nc.declare_dram_parameter("xt", [65, 4, 1024], dt.float16, False)
    wvs_d = nc.declare_dram_parameter("wvs", [128, 8, 66], dt.bfloat16, False)
    wft_d = nc.declare_dram_parameter("wft", [128, 8, 1024], dt.float16, False)
    out_d = nc.declare_dram_parameter("out", [4, 1024, 1024], dt.float16, True)

    with ExitStack() as ctx:
        tc = ctx.enter_context(tile.TileContext(nc))
        consts = ctx.enter_context(tc.tile_pool(name="consts", bufs=1))
        outp = ctx.enter_context(tc.tile_pool(name="outp", bufs=3))

        # static sbuf tensors; gpsimd memsets have no DMA deps so they run
        # from t=0 (warm first: the PE warmup depends on it)
        warm = consts.tile([128, 512], dt.float16, name="warm")
        nc.gpsimd.memset(warm[:], 0.0)
        expC = consts.tile([128, 8, 4, 128], dt.bfloat16, name="expC")
        nc.gpsimd.memset(expC[0:64], 0.0)
        nc.gpsimd.memset(expC[64:128], 0.0)

        # input DMAs: pk (xh+wqt+wkt first so G->P2->SC never wait) + xt on
        # the sync HWDGE queue; wvs + wft chunks stream on the scalar queue
        # so the N-stage consumes wft at the DMA's own cadence
        pk = consts.tile([128, 5280], dt.float16, name="pk")
        nc.sync.dma_start(out=pk[:, 0:4128], in_=pk_d[:, 0:4128])
        nc.sync.dma_start(out=pk[:, 4128:5280], in_=pk_d[:, 4128:5280])
        wvs = consts.tile([128, 8, 66], dt.bfloat16, name="wvs")
        nc.scalar.dma_start(out=wvs[:], in_=wvs_d[:, :, :])
        wft = consts.tile([128, 8, 1024], dt.float16, name="wft")
        xt = consts.tile([65, 4, 1024], dt.float16, name="xt")
        nc.sync.dma_start(out=xt[:], in_=xt_d[:, :, :])
        for q in range(4):
            nc.scalar.dma_start(
                out=wft[:, 2 * q : 2 * q + 2], in_=wft_d[:, 2 * q : 2 * q + 2, :]
            )

        xh = pk[:, 0:2080].rearrange("p (c j e) -> p c j e", c=8, j=4)
        wqt = pk[0:65, 2080:3104]
        wkt = pk[0:65, 3104:4128]
        bfh = pk[0:1, 4128:5152]
        ub = pk[0:1, 5152:5217]
        psb2 = consts.tile([65, 4, 1024], dt.float16, name="psb2")
        gsb = [consts.tile([65, 65], dt.float16, name=f"gsb{j}") for j in range(4)]
        msb = [consts.tile([128, 8, 65], dt.float16, name=f"msb{j}") for j in range(4)]
        nsb = [consts.tile([65, 1024], dt.float16, name=f"nsb{j}") for j in range(4)]
        rec = consts.tile([128, 8, 4], dt.float32, name="rec")

        # PE warmup: dense dummy matmuls run while the input DMAs land, so
        # the DVFS clock gate is already at 8/8 when real work starts. Also
        # preload the Exp activation table off the critical path.
        wexp = consts.tile([1, 16], dt.float16, name="wexp")
        nc.scalar.activation(out=wexp[:], in_=warm[0:1, 0:16], func=AF.Exp)

        # ---------------- phase 0: warmup + G (only needs xh)
        with tc.tile_pool(name="pwarm", bufs=1, space="PSUM") as pw, \
             tc.tile_pool(name="pg", bufs=1, space="PSUM") as pg:
            wps = pw.tile([128, 512], dt.float32, name="warmps", tag="pw")
            for _ in range(6):
                nc.tensor.matmul(wps[:], warm[:, 0:128], warm[:], start=True, stop=True)
            # G_j = xa_j.T @ xa_j  (65, 65)
            for j in range(4):
                gps = pg.tile([65, 65], dt.float32, name=f"gps{j}", tag="g")
                for c in range(8):
                    nc.tensor.matmul(
                        gps[:], xh[:, c, j, :], xh[:, c, j, :],
                        start=(c == 0), stop=(c == 7),
                    )
                if j % 2 == 0:
                    nc.vector.tensor_copy(out=gsb[j][:], in_=gps[:])
                else:
                    nc.scalar.copy(out=gsb[j][:], in_=gps[:])

        # ---------------- head phase: P2, scores+exp, M for all slices
        # (BOTH halves of N0 accumulate inside the M loop: N0 is fully done
        # when the heads end, out(0) starts ~7us earlier, and the PE stays
        # occupied through the window where wft is still streaming in —
        # otherwise occupancy dips and the DVFS governor halves the clock)
        with tc.tile_pool(name="pp0", bufs=1, space="PSUM") as pp0, \
             tc.tile_pool(name="pp1", bufs=1, space="PSUM") as pp1, \
             tc.tile_pool(name="psc", bufs=2, space="PSUM") as psc, \
             tc.tile_pool(name="pn0", bufs=1, space="PSUM") as pn0, \
             tc.tile_pool(name="pm", bufs=2, space="PSUM") as pm:
            # P2_j = G_j @ W~q  (65, 1024)
            for j in range(4):
                ppsa = pp0.tile([65, 512], dt.float32, name=f"pps{j}a", tag="pa")
                ppsb = pp1.tile([65, 512], dt.float32, name=f"pps{j}b", tag="pb")
                nc.tensor.matmul(ppsa[:], gsb[j][:], wqt[:, 0:512], start=True, stop=True)
                nc.tensor.matmul(ppsb[:], gsb[j][:], wqt[:, 512:1024], start=True, stop=True)
                nc.vector.tensor_copy(out=psb2[:, j, 0:512], in_=ppsa[:])
                nc.scalar.copy(out=psb2[:, j, 512:1024], in_=ppsb[:])
            # scT chunks for all 4 slices in one matmul per chunk c:
            # scp = wkt_c.T @ [P2_0 | P2_1 | P2_2 | P2_3]_c   (128, 4*128)
            for c in range(8):
                scp = psc.tile([128, 4, 128], dt.float32, name=f"scp{c}", tag="s")
                nc.tensor.matmul(
                    scp[:],
                    wkt[:, c * 128 : (c + 1) * 128],
                    psb2[:, :, c * 128 : (c + 1) * 128],
                    start=True, stop=True,
                )
                # exp of the two diagonal 64x64 blocks per slice
                nc.scalar.activation(
                    out=expC[0:64, c, :, 0:64], in_=scp[0:64, :, 0:64], func=AF.Exp
                )
                nc.scalar.activation(
                    out=expC[64:128, c, :, 64:128], in_=scp[64:128, :, 64:128], func=AF.Exp
                )
            # M~^T per (c, j): block-diag lhsT does both heads in one matmul;
            # both N0 halves accumulate chunk-by-chunk right behind it
            nsp00 = pn0.tile([65, 512], dt.float32, name="nsp00", tag="pn0a")
            nsp01 = pn0.tile([65, 512], dt.float32, name="nsp01", tag="pn0b")
            for c in range(8):
                mps = pm.tile([128, 4, 128], dt.float32, name=f"mps{c}", tag="m")
                for j in range(4):
                    nc.tensor.matmul(
                        mps[:, j, 0:66], expC[:, c, j, :], wvs[:, c, :],
                        start=True, stop=True,
                    )
                nc.vector.reciprocal(out=rec[:, c, :], in_=mps[:, :, 65])
                for j in range(4):
                    if j % 2 == 0:
                        nc.vector.tensor_scalar_mul(
                            out=msb[j][:, c, :], in0=mps[:, j, 0:65],
                            scalar1=rec[:, c, j : j + 1],
                        )
                    else:
                        nc.scalar.mul(
                            out=msb[j][:, c, :], in_=mps[:, j, 0:65],
                            mul=rec[:, c, j : j + 1],
                        )
                nc.tensor.matmul(
                    nsp00[:], msb[0][:, c, :], wft[:, c, 0:512],
                    start=(c == 0), stop=False,
                )
                nc.tensor.matmul(
                    nsp01[:], msb[0][:, c, :], wft[:, c, 512:1024],
                    start=(c == 0), stop=False,
                )
            nc.tensor.matmul(nsp00[:], ub[:], bfh[:, 0:512], start=False, stop=True)
            nc.tensor.matmul(nsp01[:], ub[:], bfh[:, 512:1024], start=False, stop=True)
            nc.scalar.copy(out=nsb[0][:, 0:512], in_=nsp00[:])
            nc.vector.tensor_copy(out=nsb[0][:, 512:1024], in_=nsp01[:])

        # ---------------- tail phase: N and out, software-striped
        with tc.tile_pool(name="pnw", bufs=1, space="PSUM") as pnw, \
             tc.tile_pool(name="pnx", bufs=1, space="PSUM") as pnx, \
             tc.tile_pool(name="po", bufs=3, space="PSUM") as po:

            def emit_N(j, nhs=(0, 1)):
                """N_j = M_j.T @ Wf.T + u64 x bf  (65, 1024), fp16 in nsb."""
                for nh in nhs:
                    pool = pnw if nh == 0 else pnx
                    nsp = pool.tile(
                        [65, 512], dt.float32, name=f"nsp{j}_{nh}", tag=f"n{nh}"
                    )
                    for c in range(8):
                        nc.tensor.matmul(
                            nsp[:], msb[j][:, c, :],
                            wft[:, c, nh * 512 : (nh + 1) * 512],
                            start=(c == 0), stop=False,
                        )
                        if c % 2 == 1:
                            yield
                    nc.tensor.matmul(
                        nsp[:], ub[:], bfh[:, nh * 512 : (nh + 1) * 512],
                        start=False, stop=True,
                    )
                    if nh == 0:
                        nc.scalar.copy(out=nsb[j][:, 0:512], in_=nsp[:])
                    else:
                        nc.vector.tensor_copy(out=nsb[j][:, 512:1024], in_=nsp[:])
                    yield

            def emit_O(j, pad=False):
                """out_j = xa_j @ N_j  (1024, 1024) fp16, streamed to HBM.
                pad=True adds WAW dummy matmuls so the tensor engine's duty
                cycle stays above the DVFS downshift threshold even when the
                psum casts pace the loop."""
                for g in range(4):
                    osb = outp.tile([128, 2, 1024], dt.float16, name=f"osb{j}_{g}", tag="osb")
                    for u in range(2):
                        c = 2 * g + u
                        ops = po.tile([128, 1024], dt.float32, name=f"ops{j}_{c}", tag="o")
                        if pad:
                            nc.tensor.matmul(
                                ops[:, 0:512], warm[0:65, 0:128], warm[0:65, 0:512],
                                start=True, stop=True,
                            )
                        nc.tensor.matmul(
                            ops[:, 0:512], xt[:, j, c * 128 : (c + 1) * 128],
                            nsb[j][:, 0:512], start=True, stop=True,
                        )
                        nc.tensor.matmul(
                            ops[:, 512:1024], xt[:, j, c * 128 : (c + 1) * 128],
                            nsb[j][:, 512:1024], start=True, stop=True,
                        )
                        if (c + j) % 2 == 0:
                            nc.vector.tensor_copy(out=osb[:, u, :], in_=ops[:])
                        else:
                            nc.scalar.copy(out=osb[:, u, :], in_=ops[:])
                        yield
                    nc.sync.dma_start(
                        out=out_d[j, g * 256 : (g + 1) * 256, :].rearrange(
                            "(u p) f -> p u f", u=2
                        ),
                        in_=osb[:],
                    )

            def drain(gen):
                for _ in gen:
                    pass

            def stripe(a, b):
                a_live, b_live = True, True
                while a_live or b_live:
                    if a_live:
                        a_live = next(a, _SENT) is not _SENT
                    if b_live:
                        b_live = next(b, _SENT) is not _SENT

            # stripe N(j+1) into out(j) so the tensor queue always has ready
            # work while nsb copies and output DMAs drain; only out(3) runs
            # bare at the end.
            stripe(emit_O(0), emit_N(1))
            stripe(emit_O(1), emit_N(2))
            stripe(emit_O(2), emit_N(3))
            drain(emit_O(3))

    nc.finalize()
    return nc


_SENT = object()


def _prep_weights(Wq, bq, Wk, bk, Wv, bv, Wf, bf):
    wqt = np.zeros((65, 1024), np.float16)
    wqt[:64] = (np.transpose(Wq, (2, 0, 1)).reshape(64, H * D) / SCALE).astype(np.float16)
    wqt[64] = (bq.reshape(H * D) / SCALE).astype(np.float16)
    wkt = np.zeros((65, 1024), np.float16)
    wkt[:64] = np.transpose(Wk, (2, 0, 1)).reshape(64, H * D).astype(np.float16)
    wkt[64] = bk.reshape(H * D).astype(np.float16)
    wva_h = np.zeros((64, 16, 66), ml_dtypes.bfloat16)
    wva_h[:, :, :64] = np.transpose(Wv, (1, 0, 2)).astype(ml_dtypes.bfloat16)
    wva_h[:, :, 64] = bv.T.astype(ml_dtypes.bfloat16)
    wva_h[:, :, 65] = 1.0
    # [128, 8, 66]: rows 0:64 = head 2c, rows 64:128 = head 2c+1 (chunk c)
    wvs = np.empty((128, 8, 66), ml_dtypes.bfloat16)
    wvs[0:64] = wva_h[:, 0::2, :]
    wvs[64:128] = wva_h[:, 1::2, :]
    wft = np.ascontiguousarray(
        Wf.T.reshape(8, 128, 1024).transpose(1, 0, 2)
    ).astype(np.float16)
    # packed weight part of pk (cols 2080:5280): wqt | wkt | bfh+ub
    pkw = np.zeros((128, 3200), np.float16)
    pkw[0:65, 0:1024] = wqt
    pkw[0:65, 1024:2048] = wkt
    pkw[0, 2048:3072] = bf.astype(np.float16)
    pkw[0, 3072 + 64] = 1.0  # ub one-hot at col 64
    return pkw, wvs, wft


def _prep_x(xs):
    """xs (1024, 256) f32 -> xh (128, 8, 4, 65) fp16 with ones col,
    xt (65, 4, 1024) fp16 with ones row."""
    x16 = xs.astype(np.float16)
    xh = np.ones((128, 8, 4, 65), np.float16)
    xh[:, :, :, :64] = x16.reshape(8, 128, 4, 64).transpose(1, 0, 2, 3)
    xt = np.ones((65, 4, 1024), np.float16)
    xt[:64] = x16.reshape(1024, 4, 64).transpose(2, 1, 0)
    return xh, xt


def _run(inputs, trace=False, tmpdir=None):
    from concourse.bass_utils import run_bass_kernel_spmd

    if "nc" not in _CACHE:
        _CACHE["nc"] = _build_nc()
    nc = _CACHE["nc"]

    x = np.ascontiguousarray(np.asarray(inputs["x"]), dtype=np.float32)
    pkw, wvs, wft = _prep_weights(
        *(np.asarray(inputs[k], dtype=np.float32) for k in
          ("Wq", "bq", "Wk", "bk", "Wv", "bv", "Wf", "bf"))
    )
    common = dict(wvs=wvs, wft=wft)
    in_maps = []
    for c in range(NCORES):
        xs = np.ascontiguousarray(x[c // 4][:, (c % 4) * 256 : (c % 4 + 1) * 256])
        xhc, xtc = _prep_x(xs)
        pk = np.empty((128, 5280), np.float16)
        pk[:, 0:2080] = xhc.reshape(128, 2080)
        pk[:, 2080:5280] = pkw
        in_maps.append(dict(pk=pk, xt=xtc, **common))

    res = run_bass_kernel_spmd(
        nc, in_maps, list(range(NCORES)), trace=trace, tmpdir=tmpdir
    )
    out = np.empty((B, H, T, E), np.float32)
    for c in range(NCORES):
        out[c // 4, 4 * (c % 4) : 4 * (c % 4) + 4] = res.results[c]["out"].astype(
            np.float32
        )
    return out, res.exec_time_ns


def kernel(**inputs) -> np.ndarray:
    out, _ = _run(inputs, trace=False)
    return out
